# revision 7
# baseline (speedup 1.0000x reference)
"""Trainium2 Bass kernel for nn_DeepPatchEncoder.

Math: the reference collapses to
    out[b] = A_X[b] @ W_dense + D_const
    D_const = (A_P + W_emb) @ W_dense + b_dense
where A_X[b] is the coarse-patchify permutation of X[b] and A_P is a
permutation of the conv-branch output (conv3x3 s2 on W_emb viewed as a
[32,32,1024] image, then BN + LeakyReLU).

Sharding (zero cross-core communication):
  Core k computes output ROWS n0 in [128k, 128(k+1)) for ALL 8 batches.
  Those rows of D_const need exactly conv output channels [512k, 512k+512),
  which core k computes itself.

v2 changes vs the fp16 baseline:
  - conv branch runs in fp8e4 with MatmulPerfMode.DoubleRow (two K-tiles
    per instruction, 2x PE throughput, and half the cw/pe DMA bytes).
    pe and cw are scaled by 2^7 each on host; the BN shift t2 seed is
    scaled by 2^14 and the 2^-14 descale is folded into the lhsC add.
  - batch matmul groups (A_X rows @ W_dense) are woven INTO the conv
    phase (conv is DMA-bound at ~440GB/s demand vs ~358GB/s available);
    five groups accumulate in PSUM banks during conv and drain after
    D_rows is ready.
  - W_dense is stored nb-major so the first woven group only needs half
    of it loaded.
  - outputs are stored fp16 (host upcasts to fp32); b_dense (zeros) is
    added on host exactly.
"""

import numpy as np
import ml_dtypes

B = 8
NC = 8
IMG = 1024
N0 = 1024
D0 = 1024
BN_EPS = 1e-3
ALPHA = 0.3
SC = 128.0          # fp8 scale for pe and cw each; products carry 2^14
SEED_SC = SC * SC   # 2^14

_CACHE = {}


# ---------------------------------------------------------------- host prep

def _perms():
    # rho2: lhsC position d0'' = 128*k2 + p2 -> natural d0 = 512u+32a+16v+bh
    #   with k2 = 4u + 2v + mb, a = 8*mb + p2//16, bh = p2 % 16
    i = np.arange(1024)
    k2, p2 = i // 128, i % 128
    u, v, mb = k2 // 4, (k2 // 2) % 2, k2 % 2
    a, bh = 8 * mb + p2 // 16, p2 % 16
    rho = 512 * u + 32 * a + 16 * v + bh
    # sigma2: conv rhs column pos2 = 128*(2u+v) + 32*ocb + j0 -> in-shard
    #   channel c_loc = 128*ocb + 64u + 2j0 + v  (so lhsC slices are plain
    #   free-dim slices of the conv output)
    p = np.arange(512)
    uv, ocb, j0 = p // 128, (p // 32) % 4, p % 32
    uu, vv = uv // 2, uv % 2
    sigma = 128 * ocb + 64 * uu + 2 * j0 + vv
    return rho, sigma


def host_prep(inputs):
    f16 = np.float16
    f8 = ml_dtypes.float8_e4m3
    X = np.asarray(inputs["X"], np.float32).reshape(B, IMG, IMG)
    W_emb = np.asarray(inputs["W_emb"], np.float32)
    conv_w = np.asarray(inputs["conv_w"], np.float32)
    conv_b = np.asarray(inputs["conv_b"], np.float32)
    g = np.asarray(inputs["bn_gamma"], np.float32)
    be = np.asarray(inputs["bn_beta"], np.float32)
    mu = np.asarray(inputs["bn_mean"], np.float32)
    var = np.asarray(inputs["bn_var"], np.float32)
    W_dense = np.asarray(inputs["W_dense"], np.float32)

    rho, sigma = _perms()

    s_all = g / np.sqrt(var + BN_EPS)
    t_all = (conv_b - mu) * s_all + be

    # pe lhsT blocks for DoubleRow, per (dd, mb, ktp, two):
    # pe[dd, p, mb*1024 + ktp*256 + two*128 + (16a'+oj)]
    #   = peT[128*(2ktp+two)+p, di+2*(8mb+a'), dj+2oj]
    peT = np.zeros((N0, 33, 33), np.float32)
    peT[:, :32, :32] = W_emb.reshape(N0, 32, 32)
    peb = np.empty((9, N0, 256), np.float32)
    for dd in range(9):
        di, dj = dd // 3, dd % 3
        blk = peT[:, di:di + 31:2, dj:dj + 31:2]       # [1024, 16, 16]
        peb[dd] = blk.reshape(N0, 256)                 # 16a+oj: a = 8mb+a'
    # old free layout per dd: (kt, mb, j') = 256kt+128mb+j'
    pe_old = peb.reshape(9, 8, 128, 2, 128)            # (dd, kt, p, mb, j)
    pe_new = pe_old.reshape(9, 4, 2, 128, 2, 128).transpose(0, 3, 4, 1, 2, 5)
    pe_host = np.ascontiguousarray(
        (pe_new.reshape(9, 128, 2, 4, 2, 128) * SC)).astype(f8)

    # W_dense row-permuted by rho, nb-major: wd[nb, p, kt, oc]
    wdp = W_dense[rho, :].reshape(8, 128, 2, 512)      # (kt, p, nb, oc)
    wd_host = np.ascontiguousarray(wdp.transpose(2, 1, 0, 3)).astype(f16)

    in_maps = []
    for k in range(NC):
        ch = 512 * k + sigma
        # conv weights as matmul RHS, BN scale folded, fp8 at 2^7:
        # cw[dd, p, ktp, two, oc], contraction channel ic = 128*(2ktp+two)+p
        cws = conv_w[:, :, :, ch] * s_all[ch]          # [3,3,1024,512]
        cw = cws.reshape(9, 4, 2, 128, 512).transpose(0, 3, 1, 2, 4)
        cw = np.ascontiguousarray(
            (cw.reshape(9, 128, 4, 2, 512) * SC)).astype(f8)
        # BN shift as a K=1 seed-matmul rhs row, at 2^14 scale
        t2 = (t_all[ch] * SEED_SC).reshape(1, 512).astype(f16)
        # A_X^T for this core's row strip, rho-permuted, SBUF layout
        # [8b, 128part, 8kt, 128j]: axt[b, p, kt, j] = A_X^T[b, 128kt+p, j]
        Xs = X[:, 128 * k:128 * (k + 1), :]            # [8,128,1024]
        axt = Xs.reshape(B, 4, 32, 32, 32).transpose(0, 2, 4, 1, 3).reshape(B, 1024, 128)
        axt = axt[:, rho, :].reshape(B, 8, 128, 128).transpose(0, 2, 1, 3)
        axt = np.ascontiguousarray(axt).astype(f16)    # [8, 128, 8, 128]
        # W_emb row block transposed into lhsC layout:
        # wrows[p, 128*k2 + j] = W_emb[128k + j, rho[128*k2 + p]]
        wr = W_emb[128 * k:128 * (k + 1), :][:, rho]   # [128j, 1024d0']
        wrows = np.ascontiguousarray(
            wr.T.reshape(8, 128, 128).transpose(1, 0, 2).reshape(128, 1024)
        ).astype(f16)
        in_maps.append({
            "cw": cw, "pe": pe_host, "t2": t2, "axt": axt,
            "wrows": wrows, "wd": wd_host,
            "ones": np.ones((1, 128), np.float16),
        })
    return in_maps


# ---------------------------------------------------------------- device code

def _build():
    import concourse.tile as tile
    import concourse.mybir as mybir
    from concourse import bacc

    f32 = mybir.dt.float32
    f16 = mybir.dt.float16
    f8 = mybir.dt.float8e4
    Alu = mybir.AluOpType
    DR = mybir.MatmulPerfMode.DoubleRow

    nc = bacc.Bacc("TRN2", target_bir_lowering=False, debug=False)

    cw_d = nc.dram_tensor("cw", [9, 128, 4, 2, 512], f8, kind="ExternalInput").ap()
    pe_d = nc.dram_tensor("pe", [9, 128, 2, 4, 2, 128], f8, kind="ExternalInput").ap()
    t2_d = nc.dram_tensor("t2", [1, 512], f16, kind="ExternalInput").ap()
    axt_d = nc.dram_tensor("axt", [8, 128, 8, 128], f16, kind="ExternalInput").ap()
    wrows_d = nc.dram_tensor("wrows", [128, 1024], f16, kind="ExternalInput").ap()
    wd_d = nc.dram_tensor("wd", [2, 128, 8, 512], f16, kind="ExternalInput").ap()
    ones_d = nc.dram_tensor("ones", [1, 128], f16, kind="ExternalInput").ap()
    out_d = nc.dram_tensor("out", [8, 128, 1024], f16, kind="ExternalOutput").ap()

    with tile.TileContext(nc) as tc:
        with (
            tc.tile_pool(name="kpool", bufs=1) as kpool,
            tc.tile_pool(name="cwpool", bufs=3) as cwpool,
            tc.tile_pool(name="zpool", bufs=2) as zpool,
            tc.tile_pool(name="opool", bufs=4) as opool,
            tc.tile_pool(name="psB", bufs=6, space="PSUM") as psB,
        ):
            # persistent SBUF tensors
            pe_sb = kpool.tile([128, 9, 2, 4, 2, 128], f8, tag="pe")
            t2_sb = kpool.tile([1, 512], f16, tag="t2")
            Y2 = kpool.tile([128, 1024], f16, tag="Y2")    # lrelu out @2^14
            lhsC = kpool.tile([128, 1024], f16, tag="lhsC")
            drows = kpool.tile([128, 1024], f16, tag="drows")
            wrows_sb = kpool.tile([128, 1024], f16, tag="wrows")
            ones_sb = kpool.tile([1, 128], f16, tag="ones")
            wd_sb = kpool.tile([128, 2, 8, 512], f16, tag="wd")
            axt_sb = kpool.tile([128, 8, 8, 128], f16, tag="axt")

            fp_tiles = {}

            def bg_open(b, nb):
                fp = psB.tile([128, 512], f32, tag="fp", name=f"fp{b}_{nb}")
                fp_tiles[(b, nb)] = fp
                for kt in range(8):
                    nc.tensor.matmul(
                        fp[:], axt_sb[:, b, kt], wd_sb[:, nb, kt],
                        start=(kt == 0), stop=(kt == 7))
                return fp

            def bg_drain(b, nb, ring="scalar"):
                fp = fp_tiles.pop((b, nb))
                ot = opool.tile([128, 512], f16, tag="ot", name=f"ot{b}_{nb}")
                nc.vector.tensor_tensor(
                    ot[:], fp[:], drows[:, 512 * nb:512 * (nb + 1)], Alu.add)
                eng = nc.scalar if ring == "scalar" else nc.sync
                eng.dma_start(out_d[b][:, 512 * nb:512 * (nb + 1)], ot[:])

            # extra sync-ring DMAs woven after each conv group's cw/pe, in
            # exact queue (= consumption) order
            def dma_axt(b):
                nc.sync.dma_start(axt_sb[:, b], axt_d[b])

            def dma_wd(nb):
                nc.sync.dma_start(wd_sb[:, nb], wd_d[nb])

            extras = {
                1: lambda: (dma_wd(0), dma_axt(0), dma_axt(1)),
                2: lambda: dma_axt(2),
                3: lambda: dma_axt(3),
                4: lambda: dma_axt(4),
                6: lambda: nc.sync.dma_start(wrows_sb[:], wrows_d[:]),
                7: lambda: dma_axt(5),
                8: lambda: (dma_axt(6), dma_axt(7), dma_wd(1)),
            }
            weave = {2: [(0, 0), (1, 0)], 4: [(2, 0), (3, 0)]}

            # ---------------- conv in fp8 DoubleRow; BN scale folded into cw
            # on host; BN shift t2 (at 2^14) seeded via K=1 fp16 matmul.
            with tc.tile_pool(name="psA", bufs=1, space="PSUM") as psA:
                y2ps = [psA.tile([128, 512], f32, tag=f"y2{mb}", name=f"y2ps{mb}")
                        for mb in range(2)]
                nc.sync.dma_start(t2_sb[:], t2_d[:])
                nc.sync.dma_start(ones_sb[:], ones_d[:])
                for mb in range(2):
                    nc.tensor.matmul(
                        y2ps[mb][:], ones_sb[:], t2_sb[:],
                        start=True, stop=False)
                # warmup: ramp the PE clock (pstate) while the first conv
                # tiles stream in; results are discarded.
                warm = psB.tile([128, 512], f32, tag="fp", name="warm")
                for _ in range(7):
                    nc.tensor.matmul(
                        warm[:], ones_sb[:], t2_sb[:], start=True, stop=True)

                def epi_half(mb):
                    # z = psum; Y2half = max(alpha*z, z) in fp16 at 2^14;
                    # then lhsC k2-slices for this half (needs wrows).
                    z = zpool.tile([128, 512], f32, tag="z")
                    nc.vector.tensor_copy(z[:], y2ps[mb][:])
                    nc.vector.scalar_tensor_tensor(
                        Y2[:, 512 * mb:512 * (mb + 1)],
                        z[:], ALPHA, z[:], Alu.mult, Alu.max)
                    for k2 in range(mb, 8, 2):
                        u, v = k2 // 4, (k2 // 2) % 2
                        s0 = 512 * mb + 128 * (2 * u + v)
                        nc.vector.scalar_tensor_tensor(
                            lhsC[:, 128 * k2:128 * (k2 + 1)],
                            Y2[:, s0:s0 + 128], 1.0 / SEED_SC,
                            wrows_sb[:, 128 * k2:128 * (k2 + 1)],
                            Alu.mult, Alu.add)

                for dd in range(9):
                    cw_t = cwpool.tile([128, 4, 2, 512], f8, tag="cw")
                    if dd == 0:
                        nc.sync.dma_start(pe_sb[:, dd], pe_d[dd])
                        nc.sync.dma_start(cw_t[:, 0:2], cw_d[dd][:, 0:2])
                        nc.sync.dma_start(cw_t[:, 2:4], cw_d[dd][:, 2:4])
                    else:
                        nc.sync.dma_start(cw_t[:], cw_d[dd])
                        nc.sync.dma_start(pe_sb[:, dd], pe_d[dd])
                    if dd in extras:
                        extras[dd]()
                    for mb in range(2):
                        for ktp in range(4):
                            nc.tensor.matmul(
                                y2ps[mb][:], pe_sb[:, dd, mb, ktp],
                                cw_t[:, ktp],
                                start=False,
                                stop=(dd == 8 and ktp == 3),
                                perf_mode=DR)
                        if dd == 8:
                            epi_half(mb)
                    for w in weave.get(dd, []):
                        bg_open(*w)

            # ---------------- bg4/bg5 cover the lhsC latency, then D_rows
            with tc.tile_pool(name="psD", bufs=1, space="PSUM") as psD:
                bg_open(4, 0)
                bg_open(5, 0)
                dps = psD.tile([128, 1024], f32, tag="dps")
                for nb in range(2):
                    o = dps[:, 512 * nb:512 * (nb + 1)]
                    for kt in range(8):
                        nc.tensor.matmul(
                            o, lhsC[:, 128 * kt:128 * (kt + 1)],
                            wd_sb[:, nb, kt],
                            start=(kt == 0), stop=(kt == 7))
                    nc.vector.tensor_copy(
                        drows[:, 512 * nb:512 * (nb + 1)], o)

                # drain held groups, then stream the rest
                for b in range(6):
                    bg_drain(b, 0, ring="scalar" if b % 2 == 0 else "sync")
                for b in (6, 7):
                    bg_open(b, 0)
                    bg_drain(b, 0, ring="scalar" if b % 2 == 0 else "sync")
                for b in range(8):
                    bg_open(b, 1)
                    bg_drain(b, 1, ring="scalar" if b % 2 == 0 else "sync")

    nc.compile()
    return nc


def get_nc():
    if "nc" not in _CACHE:
        _CACHE["nc"] = _build()
    return _CACHE["nc"]


# ---------------------------------------------------------------- entry points

def run(inputs, trace=False, **kwargs):
    from concourse.bass_utils import run_bass_kernel_spmd
    nc = get_nc()
    in_maps = host_prep(inputs)
    res = run_bass_kernel_spmd(nc, in_maps, list(range(NC)), trace=trace, **kwargs)
    b_dense = np.asarray(inputs["b_dense"], np.float32)
    out = np.empty((B, N0, D0), np.float32)
    for k in range(NC):
        out[:, 128 * k:128 * (k + 1), :] = res.results[k]["out"].astype(np.float32)
    out += b_dense
    return out, res


def kernel(**inputs):
    out, _ = run(inputs)
    return out


# revision 12
# speedup vs baseline: 1.0424x; 1.0424x over previous
"""Trainium2 Bass kernel for nn_DeepPatchEncoder.

Math: the reference collapses to
    out[b] = A_X[b] @ W_dense + D_const
    D_const = (A_P + W_emb) @ W_dense + b_dense
where A_X[b] is the coarse-patchify permutation of X[b] and A_P is a
permutation of the conv-branch output (conv3x3 s2 on W_emb viewed as a
[32,32,1024] image, then BN + LeakyReLU).

Sharding (zero cross-core communication):
  Core k computes output ROWS n0 in [128k, 128(k+1)) for ALL 8 batches.
  Those rows of D_const need exactly conv output channels [512k, 512k+512),
  which core k computes itself.

v2 changes vs the fp16 baseline:
  - conv branch runs in fp8e4 with MatmulPerfMode.DoubleRow (two K-tiles
    per instruction, 2x PE throughput, and half the cw/pe DMA bytes).
    pe and cw are scaled by 2^7 each on host; the BN shift t2 seed is
    scaled by 2^14 and the 2^-14 descale is folded into the lhsC add.
  - batch matmul groups (A_X rows @ W_dense) are woven INTO the conv
    phase (conv is DMA-bound at ~440GB/s demand vs ~358GB/s available);
    five groups accumulate in PSUM banks during conv and drain after
    D_rows is ready.
  - W_dense is stored nb-major so the first woven group only needs half
    of it loaded.
  - outputs are stored fp16 (host upcasts to fp32); b_dense (zeros) is
    added on host exactly.
"""

import numpy as np
import ml_dtypes

B = 8
NC = 8
IMG = 1024
N0 = 1024
D0 = 1024
BN_EPS = 1e-3
ALPHA = 0.3
SC = 128.0          # fp8 scale for pe and cw each; products carry 2^14
SEED_SC = SC * SC   # 2^14

_CACHE = {}


# ---------------------------------------------------------------- host prep

def _perms():
    # rho2: lhsC position d0'' = 128*k2 + p2 -> natural d0 = 512u+32a+16v+bh
    #   with k2 = 4u + 2v + mb, a = 8*mb + p2//16, bh = p2 % 16
    i = np.arange(1024)
    k2, p2 = i // 128, i % 128
    u, v, mb = k2 // 4, (k2 // 2) % 2, k2 % 2
    a, bh = 8 * mb + p2 // 16, p2 % 16
    rho = 512 * u + 32 * a + 16 * v + bh
    # sigma2: conv rhs column pos2 = 128*(2u+v) + 32*ocb + j0 -> in-shard
    #   channel c_loc = 128*ocb + 64u + 2j0 + v  (so lhsC slices are plain
    #   free-dim slices of the conv output)
    p = np.arange(512)
    uv, ocb, j0 = p // 128, (p // 32) % 4, p % 32
    uu, vv = uv // 2, uv % 2
    sigma = 128 * ocb + 64 * uu + 2 * j0 + vv
    return rho, sigma


def host_prep(inputs):
    f16 = np.float16
    f8 = ml_dtypes.float8_e4m3
    X = np.asarray(inputs["X"], np.float32).reshape(B, IMG, IMG)
    W_emb = np.asarray(inputs["W_emb"], np.float32)
    conv_w = np.asarray(inputs["conv_w"], np.float32)
    conv_b = np.asarray(inputs["conv_b"], np.float32)
    g = np.asarray(inputs["bn_gamma"], np.float32)
    be = np.asarray(inputs["bn_beta"], np.float32)
    mu = np.asarray(inputs["bn_mean"], np.float32)
    var = np.asarray(inputs["bn_var"], np.float32)
    W_dense = np.asarray(inputs["W_dense"], np.float32)

    rho, sigma = _perms()

    s_all = g / np.sqrt(var + BN_EPS)
    t_all = (conv_b - mu) * s_all + be

    # pe lhsT blocks for DoubleRow, per (dd, mb, ktp, two):
    # pe[dd, p, mb*1024 + ktp*256 + two*128 + (16a'+oj)]
    #   = peT[128*(2ktp+two)+p, di+2*(8mb+a'), dj+2oj]
    peT = np.zeros((N0, 33, 33), np.float32)
    peT[:, :32, :32] = W_emb.reshape(N0, 32, 32)
    peb = np.empty((9, N0, 256), np.float32)
    for dd in range(9):
        di, dj = dd // 3, dd % 3
        blk = peT[:, di:di + 31:2, dj:dj + 31:2]       # [1024, 16, 16]
        peb[dd] = blk.reshape(N0, 256)                 # 16a+oj: a = 8mb+a'
    # old free layout per dd: (kt, mb, j') = 256kt+128mb+j'
    pe_old = peb.reshape(9, 8, 128, 2, 128)            # (dd, kt, p, mb, j)
    pe_new = pe_old.reshape(9, 4, 2, 128, 2, 128).transpose(0, 3, 4, 1, 2, 5)
    pe_host = np.ascontiguousarray(
        (pe_new.reshape(9, 128, 2, 4, 2, 128) * SC)).astype(f8)

    # W_dense row-permuted by rho, nb-major: wd[nb, p, kt, oc]
    wdp = W_dense[rho, :].reshape(8, 128, 2, 512)      # (kt, p, nb, oc)
    wd_host = np.ascontiguousarray(wdp.transpose(2, 1, 0, 3)).astype(f16)

    in_maps = []
    for k in range(NC):
        ch = 512 * k + sigma
        # conv weights as matmul RHS, BN scale folded, fp8 at 2^7:
        # cw[dd, p, ktp, two, oc], contraction channel ic = 128*(2ktp+two)+p
        cws = conv_w[:, :, :, ch] * s_all[ch]          # [3,3,1024,512]
        cw = cws.reshape(9, 4, 2, 128, 512).transpose(0, 3, 1, 2, 4)
        cw = np.ascontiguousarray(
            (cw.reshape(9, 128, 4, 2, 512) * SC)).astype(f8)
        # BN shift as a K=1 seed-matmul rhs row, at 2^14 scale
        t2 = (t_all[ch] * SEED_SC).reshape(1, 512).astype(f16)
        # A_X^T for this core's row strip, rho-permuted, SBUF layout
        # [8b, 128part, 8kt, 128j]: axt[b, p, kt, j] = A_X^T[b, 128kt+p, j]
        Xs = X[:, 128 * k:128 * (k + 1), :]            # [8,128,1024]
        axt = Xs.reshape(B, 4, 32, 32, 32).transpose(0, 2, 4, 1, 3).reshape(B, 1024, 128)
        axt = axt[:, rho, :].reshape(B, 8, 128, 128).transpose(0, 2, 1, 3)
        axt = np.ascontiguousarray(axt).astype(f16)    # [8, 128, 8, 128]
        # W_emb row block transposed into lhsC layout:
        # wrows[p, 128*k2 + j] = W_emb[128k + j, rho[128*k2 + p]]
        wr = W_emb[128 * k:128 * (k + 1), :][:, rho]   # [128j, 1024d0']
        wrows = np.ascontiguousarray(
            wr.T.reshape(8, 128, 128).transpose(1, 0, 2).reshape(128, 1024)
        ).astype(f16)
        in_maps.append({
            "cw": cw, "pe": pe_host, "t2": t2, "axt": axt,
            "wrows": wrows, "wd": wd_host,
            "ones": np.ones((1, 128), np.float16),
        })
    return in_maps


# ---------------------------------------------------------------- device code

def _build():
    import concourse.tile as tile
    import concourse.mybir as mybir
    from concourse import bacc

    f32 = mybir.dt.float32
    f16 = mybir.dt.float16
    f8 = mybir.dt.float8e4
    Alu = mybir.AluOpType
    DR = mybir.MatmulPerfMode.DoubleRow

    nc = bacc.Bacc("TRN2", target_bir_lowering=False, debug=False)

    cw_d = nc.dram_tensor("cw", [9, 128, 4, 2, 512], f8, kind="ExternalInput").ap()
    pe_d = nc.dram_tensor("pe", [9, 128, 2, 4, 2, 128], f8, kind="ExternalInput").ap()
    t2_d = nc.dram_tensor("t2", [1, 512], f16, kind="ExternalInput").ap()
    axt_d = nc.dram_tensor("axt", [8, 128, 8, 128], f16, kind="ExternalInput").ap()
    wrows_d = nc.dram_tensor("wrows", [128, 1024], f16, kind="ExternalInput").ap()
    wd_d = nc.dram_tensor("wd", [2, 128, 8, 512], f16, kind="ExternalInput").ap()
    ones_d = nc.dram_tensor("ones", [1, 128], f16, kind="ExternalInput").ap()
    out_d = nc.dram_tensor("out", [8, 128, 1024], f16, kind="ExternalOutput").ap()

    with tile.TileContext(nc) as tc:
        with (
            tc.tile_pool(name="kpool", bufs=1) as kpool,
            tc.tile_pool(name="cwpool", bufs=3) as cwpool,
            tc.tile_pool(name="zpool", bufs=2) as zpool,
            tc.tile_pool(name="opool", bufs=4) as opool,
            tc.tile_pool(name="psB", bufs=6, space="PSUM") as psB,
        ):
            # persistent SBUF tensors
            pe_sb = kpool.tile([128, 9, 2, 4, 2, 128], f8, tag="pe")
            t2_sb = kpool.tile([1, 512], f16, tag="t2")
            Y2 = kpool.tile([128, 1024], f16, tag="Y2")    # lrelu out @2^14
            lhsC = kpool.tile([128, 1024], f16, tag="lhsC")
            drows = kpool.tile([128, 1024], f16, tag="drows")
            wrows_sb = kpool.tile([128, 1024], f16, tag="wrows")
            ones_sb = kpool.tile([1, 128], f16, tag="ones")
            wd_sb = kpool.tile([128, 2, 8, 512], f16, tag="wd")
            axt_sb = kpool.tile([128, 8, 8, 128], f16, tag="axt")

            fp_tiles = {}

            def bg_open(b, nb):
                fp = psB.tile([128, 512], f32, tag="fp", name=f"fp{b}_{nb}")
                fp_tiles[(b, nb)] = fp
                for kt in range(8):
                    nc.tensor.matmul(
                        fp[:], axt_sb[:, b, kt], wd_sb[:, nb, kt],
                        start=(kt == 0), stop=(kt == 7))
                return fp

            def bg_drain(b, nb):
                fp = fp_tiles.pop((b, nb))
                ot = opool.tile([128, 512], f16, tag="ot", name=f"ot{b}_{nb}")
                nc.vector.tensor_tensor(
                    ot[:], fp[:], drows[:, 512 * nb:512 * (nb + 1)], Alu.add)
                nc.scalar.dma_start(out_d[b][:, 512 * nb:512 * (nb + 1)], ot[:])

            # extra sync-ring DMAs woven after each conv group's cw/pe, in
            # exact queue (= consumption) order
            def dma_axt(b):
                nc.sync.dma_start(axt_sb[:, b], axt_d[b])

            def dma_wd(nb):
                nc.sync.dma_start(wd_sb[:, nb], wd_d[nb])

            extras = {
                2: lambda: (dma_wd(0), dma_axt(0), dma_axt(1)),
                4: lambda: (dma_axt(2), dma_axt(3)),
                6: lambda: nc.sync.dma_start(wrows_sb[:], wrows_d[:]),
                7: lambda: (dma_axt(4), dma_axt(5)),
                8: lambda: (dma_axt(6), dma_axt(7), dma_wd(1)),
            }
            weave = {2: [(0, 0), (1, 0)], 4: [(2, 0), (3, 0)]}

            # ---------------- conv in fp8 DoubleRow; BN scale folded into cw
            # on host; BN shift t2 (at 2^14) seeded via K=1 fp16 matmul.
            with tc.tile_pool(name="psA", bufs=1, space="PSUM") as psA:
                y2ps = [psA.tile([128, 512], f32, tag=f"y2{mb}", name=f"y2ps{mb}")
                        for mb in range(2)]
                nc.sync.dma_start(t2_sb[:], t2_d[:])
                nc.sync.dma_start(ones_sb[:], ones_d[:])
                for mb in range(2):
                    nc.tensor.matmul(
                        y2ps[mb][:], ones_sb[:], t2_sb[:],
                        start=True, stop=False)
                # warmup: ramp the PE clock (pstate) while the first conv
                # tiles stream in; results are discarded.
                warm = psB.tile([128, 512], f32, tag="fp", name="warm")
                for _ in range(3):
                    nc.tensor.matmul(
                        warm[:], ones_sb[:], t2_sb[:], start=True, stop=True)

                def epi_half(mb):
                    # z = psum; Y2half = max(alpha*z, z) in fp16 at 2^14;
                    # then lhsC k2-slices for this half (needs wrows).
                    z = zpool.tile([128, 512], f32, tag="z")
                    nc.vector.tensor_copy(z[:], y2ps[mb][:])
                    nc.vector.scalar_tensor_tensor(
                        Y2[:, 512 * mb:512 * (mb + 1)],
                        z[:], ALPHA, z[:], Alu.mult, Alu.max)
                    for k2 in range(mb, 8, 2):
                        u, v = k2 // 4, (k2 // 2) % 2
                        s0 = 512 * mb + 128 * (2 * u + v)
                        nc.vector.scalar_tensor_tensor(
                            lhsC[:, 128 * k2:128 * (k2 + 1)],
                            Y2[:, s0:s0 + 128], 1.0 / SEED_SC,
                            wrows_sb[:, 128 * k2:128 * (k2 + 1)],
                            Alu.mult, Alu.add)

                for dd in range(9):
                    cw_t = cwpool.tile([128, 4, 2, 512], f8, tag="cw")
                    if dd == 0:
                        nc.sync.dma_start(pe_sb[:, dd], pe_d[dd])
                        nc.sync.dma_start(cw_t[:, 0:2], cw_d[dd][:, 0:2])
                        nc.sync.dma_start(cw_t[:, 2:4], cw_d[dd][:, 2:4])
                    else:
                        nc.sync.dma_start(cw_t[:], cw_d[dd])
                        nc.sync.dma_start(pe_sb[:, dd], pe_d[dd])
                    if dd in extras:
                        extras[dd]()
                    for mb in range(2):
                        for ktp in range(4):
                            nc.tensor.matmul(
                                y2ps[mb][:], pe_sb[:, dd, mb, ktp],
                                cw_t[:, ktp],
                                start=False,
                                stop=(dd == 8 and ktp == 3),
                                perf_mode=DR)
                        if dd == 8:
                            epi_half(mb)
                    for w in weave.get(dd, []):
                        bg_open(*w)

            # ---------------- bg4/bg5 cover the lhsC latency, then D_rows
            with tc.tile_pool(name="psD", bufs=1, space="PSUM") as psD:
                bg_open(4, 0)
                bg_open(5, 0)
                dps = psD.tile([128, 1024], f32, tag="dps")
                for nb in range(2):
                    o = dps[:, 512 * nb:512 * (nb + 1)]
                    for kt in range(8):
                        nc.tensor.matmul(
                            o, lhsC[:, 128 * kt:128 * (kt + 1)],
                            wd_sb[:, nb, kt],
                            start=(kt == 0), stop=(kt == 7))
                    nc.vector.tensor_copy(
                        drows[:, 512 * nb:512 * (nb + 1)], o)

                # drain held groups, then stream the rest
                for b in range(6):
                    bg_drain(b, 0)
                for b in (6, 7):
                    bg_open(b, 0)
                    bg_drain(b, 0)
                for b in range(8):
                    bg_open(b, 1)
                    bg_drain(b, 1)

    nc.compile()
    return nc


def get_nc():
    if "nc" not in _CACHE:
        _CACHE["nc"] = _build()
    return _CACHE["nc"]


# ---------------------------------------------------------------- entry points

def run(inputs, trace=False, **kwargs):
    from concourse.bass_utils import run_bass_kernel_spmd
    nc = get_nc()
    in_maps = host_prep(inputs)
    res = run_bass_kernel_spmd(nc, in_maps, list(range(NC)), trace=trace, **kwargs)
    b_dense = np.asarray(inputs["b_dense"], np.float32)
    out = np.empty((B, N0, D0), np.float32)
    for k in range(NC):
        out[:, 128 * k:128 * (k + 1), :] = res.results[k]["out"].astype(np.float32)
    out += b_dense
    return out, res


def kernel(**inputs):
    out, _ = run(inputs)
    return out
